# revision 60
# baseline (speedup 1.0000x reference)
"""Trainium2 Bass kernel for 16-head causal multi-head attention.

Problem: B=2, S=2048, D=1024, H=16 (head dim 64), causal mask.
    out = softmax((XqWq+bq)(XkWk+bk)^T / 8, causal) (XvWv+bv) Wo + bo

Sharding: tensor-parallel over heads. Each of the 8 cores owns 2 heads:
Wq/Wk/Wv column-sliced (128 cols), Wo row-sliced (128 rows). Each core
computes its heads end-to-end and produces a partial output (ctx_c @ Wo_c);
the host sums the 8 partials and adds (bv @ Wo + bo).

Device-side structure (per core):
  - X^T (features-major) fp16 inputs so projection matmuls contract over
    features on partitions with no device transposes.
  - Scores computed transposed, S^T[k, q] = K @ Q^T; exp has no max
    subtraction (scores/8 ~ N(0,1)), so the softmax denominator comes from a
    ones column appended to V.
  - PV is computed q-major ("orientation B"): stationary = P^T q-subtile
    [k=128, q=128], moving = V(+ones) [k=128, 65] -> out [q=128, 65].  All
    128 output partitions are used (PE cost model charges free size only),
    halving PV PE time vs the dk-major layout, and normalisation becomes a
    per-partition scalar multiply fused into the PSUM eviction.
  - ctx[q, dk] for both heads packed side by side [q, 128] is transposed
    back to dk-major with one PE transpose per q-subtile, then Wo applies.
  - Causal diagonal tiles use shifted exp windows + a triangular 0/1
    multiply on the DVE.
  - Emission interleaves chunk j's attention tiles with chunk j+1's
    projection matmuls and chunk j-1's transpose/Wo matmuls so the in-order
    PE never starves while the ACT engine runs exp.
"""

import math

import numpy as np

# Full-problem constants
B, S, D, H = 2, 2048, 1024, 16
DK = D // H  # 64
NCORES = 8
HPC = H // NCORES  # heads per core
P = 128
QC = 512  # tokens per attention q-chunk / projection chunk

_PROGRAM_CACHE = {}
TRACE = False
LAST = {}
DBG = False


# ---------------------------------------------------------------------------
# Device program
# ---------------------------------------------------------------------------

def _mha_body(ctx, tc, io, s, d, b):
    import concourse.bass as bass
    from concourse import mybir

    F16 = mybir.dt.float16
    F32 = mybir.dt.float32
    F8 = mybir.dt.float8e4
    DR = mybir.MatmulPerfMode.DoubleRow
    Exp = mybir.ActivationFunctionType.Exp
    Identity = mybir.ActivationFunctionType.Identity

    nc = tc.nc
    nch = s // QC       # q chunks per sequence (4)
    kpc = QC // P       # k tiles per chunk (4)
    nf = d // P         # feature tiles (8)
    nj = b * nch        # total chunks (8)

    x2 = {"q": io["xq2"], "k": io["xk2"], "v": io["xv2"]}
    w2 = {"q": io["wq2"], "k": io["wk2"], "v": io["wv2"]}
    wo = io["wo"]
    bq, bk = io["bq"], io["bk"]
    tri, idn = io["tri"], io["idn"]
    out_t = io["out_t"]

    consts = ctx.enter_context(tc.tile_pool(name="consts", bufs=1))
    xs = ctx.enter_context(tc.tile_pool(name="xs", bufs=1))
    persist = ctx.enter_context(tc.tile_pool(name="persist", bufs=1))
    qkpool = ctx.enter_context(tc.tile_pool(name="qkpool", bufs=2))
    pts = ctx.enter_context(tc.tile_pool(name="pts", bufs=24))
    norms = ctx.enter_context(tc.tile_pool(name="norms", bufs=2))
    wouts = ctx.enter_context(tc.tile_pool(name="wouts", bufs=3))
    pspool = ctx.enter_context(tc.tile_pool(name="ps", bufs=1, space="PSUM"))

    # PSUM bank map (8 banks):
    #   swA (2 banks) / swB (2 banks): double-buffered wide score tiles
    #   pv0 / pv1: PV accumulators per head [128 q, 4 qs, 65]
    #   proj: projection outputs [128, 512]
    #   wob: ctx transposes [128, 4, 128] f16, then Wo outputs [128, 512]

    # ---- constants (critical-path order: Q path first) -------------------
    # fp8 weights/X: each [d, *] object is stored hi (fp8(32W) / fp8(X)) and
    # lo (fp8 of the quantisation residual); projections run 3 DoubleRow
    # terms hi*hi + hi*xlo + wlo*hi, all at scale 32, evicted with 1/32.
    nfp = nf // 2  # feature-tile PAIRS per DoubleRow matmul (4)
    c0x = {}
    w_sb = {}

    def load_w(nm):
        t = consts.tile([P, 2, nfp, 2, P], F8, tag=f"w{nm}")
        nc.sync.dma_start(
            t[:], w2[nm].rearrange("p (w fp g m) -> p w fp g m", w=2, fp=nfp, g=2))
        w_sb[(nm, 0)] = t[:, 0]
        w_sb[(nm, 1)] = t[:, 1]

    def load_x(bb, nm, tag, lo, split=False, eng=None):
        """One [128, 2(hi/lo), nfp, 2, 512] fp8 tile, one DMA (two when
        split: hi first so the main projection term can start early)."""
        eng = eng or nc.sync
        t = xs.tile([P, 2, nfp, 2, QC], F8, tag=tag, name=tag)
        src_r = x2[nm].rearrange("(w fp g p) t -> p w fp g t", w=2, g=2, p=P, fp=nfp)
        if split:
            eng.dma_start(t[:, 0], src_r[:, 0, :, :, lo:lo + QC])
            eng.dma_start(t[:, 1], src_r[:, 1, :, :, lo:lo + QC])
        else:
            eng.dma_start(t[:], src_r[:, :, :, :, lo:lo + QC])
        return t

    # startup: exactly the tensors each projection term needs, in need
    # order, spread over SP/ACT/DVE issue queues so sequencer issue costs
    # overlap (transfers still serialize on the DMA engines)
    def load_x_half(nm, tag, lo, which):
        t = xs.tile([P, 2, nfp, 2, QC], F8, tag=tag, name=tag)
        src_r = x2[nm].rearrange("(w fp g p) t -> p w fp g t", w=2, g=2,
                                 p=P, fp=nfp)
        nc.sync.dma_start(t[:, which], src_r[:, which, :, :, lo:lo + QC])
        return t

    load_w("q")
    bq_sb = consts.tile([P, 1], F32, tag="bq")
    nc.gpsimd.dma_start(bq_sb[:], bq[:, :])
    c0q = load_x_half("q", "c0q0", 0, 0)   # hi halves first: the main
    load_w("k")                            # fp8 term of Q AND K can start
    c0k = load_x_half("k", "c0k0", 0, 0)   # before any lo half arrives
    bk_sb = consts.tile([P, 1], F32, tag="bk")
    nc.gpsimd.dma_start(bk_sb[:], bk[:, :])
    load_w("v")
    nc.sync.dma_start(c0q[:, 1], x2["q"].rearrange(
        "(w fp g p) t -> p w fp g t", w=2, g=2, p=P, fp=nfp)[:, 1, :, :, 0:QC])
    nc.sync.dma_start(c0k[:, 1], x2["k"].rearrange(
        "(w fp g p) t -> p w fp g t", w=2, g=2, p=P, fp=nfp)[:, 1, :, :, 0:QC])
    c0x[(0, "q")] = c0q
    c0x[(0, "k")] = c0k
    c0x[(0, "v")] = load_x(0, "v", "c0v0", 0)

    negi_sb = consts.tile([P, P], F16, tag="negi")
    nc.sync.dma_start(negi_sb[:], tri[:, :])  # host sends -60000*I here
    idn_sb = consts.tile([P, P], F16, tag="idn")
    nc.sync.dma_start(idn_sb[:], idn[:, :])
    slt_sb = consts.tile([P, P], F16, tag="slt")
    nc.sync.dma_start(slt_sb[:], io["slt"][:, :])
    wo_sb = consts.tile([P, d], F16, tag="wo")
    nc.sync.dma_start(wo_sb[:], wo[:, :])

    # batch-0 remaining chunks: one DMA pair per (input, chunk)
    ctx_tiles = {}
    def issue_xt(bb, chunks=None):
        for cc in (chunks or range(1, nch)):
            for nm in ("q", "k", "v"):
                ctx_tiles[(bb, nm, cc)] = load_x(
                    bb, nm, f"ct{nm}{cc}", bb * s + cc * QC)

    issue_xt(0)

    def issue_c0(bb):
        # reuses batch-0's c0 tags (their readers finish in phase 0)
        for nm in ("q", "k", "v"):
            c0x[(bb, nm)] = load_x(bb, nm, f"c0{nm}0", bb * s)

    def xsl(bb, nm, which, fp, lo, hi):
        """fp8 X^T pair slice [128, 2, lo:hi] for feature-pair fp."""
        if hi <= QC:
            return c0x[(bb, nm)][:, which, fp, :, lo:hi]
        cc = lo // QC
        t = ctx_tiles[(bb, nm, cc)]
        return t[:, which, fp, :, lo - cc * QC:hi - cc * QC]

    qt_tiles = {}
    kt_tiles = {}
    v_tiles = {}

    # ---- projection emission (as filler closures) ------------------------
    def proj_steps(j, v_only=False):
        """Closures, each emitting ~1 PE matmul group for chunk j's Q/K/V."""
        bb, jj = divmod(j, nch)
        co = jj * QC
        steps = []

        def qk_proj(nm, b_sb):
            pp = pspool.tile([P, QC], F32, tag="proj", name=f"pp{nm}")
            terms = [(0, 0), (1, 0), (0, 1)]  # (w which, x which); x-lo last
            for ti, (ww, xw) in enumerate(terms):
                for fp in range(nf // 2):
                    def mm(fp=fp, pp=pp, nm=nm, ww=ww, xw=xw, ti=ti):
                        nc.tensor.matmul(pp[:], w_sb[(nm, ww)][:, fp, :, :],
                                         xsl(bb, nm, xw, fp, co, co + QC),
                                         start=(ti == 0 and fp == 0),
                                         stop=(ti == 2 and fp == nf // 2 - 1),
                                         perf_mode=DR)
                    steps.append(mm)

            def evict(pp=pp, nm=nm, b_sb=b_sb):
                if nm == "q":
                    t = qkpool.tile([P, QC], F16, tag="qt", name="qt")
                    qt_tiles[j] = t
                else:
                    # K persists for the whole batch (later chunks read it)
                    t = persist.tile([P, QC], F16, tag=f"kt{jj}",
                                     name=f"kt{jj}")
                    kt_tiles[(bb, jj)] = t
                if j < 5:
                    nc.scalar.activation(t[:], pp[:], Identity,
                                         bias=b_sb[:, 0:1], scale=1.0 / 32.0)
                else:
                    # late phases: ACT is exp-saturated; DVE does scale+bias
                    nc.vector.tensor_scalar(
                        t[:], pp[:], 1.0 / 32.0, b_sb[:, 0:1],
                        mybir.AluOpType.mult, mybir.AluOpType.add)
                if DBG and j == 0:
                    nc.sync.dma_start(io[f"dbg_{nm}t"][:, :], t[:])
            steps.append(evict)

        if not v_only:
            qk_proj("q", bq_sb)
            qk_proj("k", bk_sb)

        # V: token-major [tok, dk] per 128-token tile, both heads + ones col.
        # PSUM accumulation groups within a bank must be sequential on HW:
        # complete each t4 region's f-loop before starting the next region.
        ppv = pspool.tile([P, QC], F32, tag="proj", name="ppv")
        vterms = [(0, 0), (0, 1), (1, 0)]  # (x which, w which); x-lo last
        for t4 in range(kpc):
            def mmv(t4=t4, ppv=ppv):
                for ti, (xw, ww) in enumerate(vterms):
                    for fp in range(nf // 2):
                        nc.tensor.matmul(
                            ppv[:, t4 * P:(t4 + 1) * P],
                            xsl(bb, "v", xw, fp, co + t4 * P,
                                co + (t4 + 1) * P),
                            w_sb[("v", ww)][:, fp, :, :],
                            start=(ti == 0 and fp == 0),
                            stop=(ti == 2 and fp == nf // 2 - 1),
                            perf_mode=DR)
            steps.append(mmv)

        def vevict(ppv=ppv, bb=bb, jj=jj):
            for t4 in range(kpc):
                kt = jj * kpc + t4
                vt = persist.tile([P, 130], F16, tag=f"v{kt}", name=f"v{kt}")
                nc.vector.tensor_scalar_mul(
                    vt.rearrange("p (h x) -> p h x", h=2)[:, :, 0:64],
                    ppv[:, t4 * P:(t4 + 1) * P].rearrange(
                        "p (h x) -> p h x", h=2), 1.0 / 32.0)
                nc.vector.memset(vt[:, 64:65], 1.0)
                if DBG and bb == 0 and kt == 0:
                    nc.sync.dma_start(io["dbg_vt"][:, :], vt[:])
                v_tiles[(bb, kt)] = vt
        steps.append(vevict)
        return steps

    # ---- postprocess (norm + transpose + Wo) for a finished chunk --------
    def postproc_prologue(j, pv):
        """Non-PE: reciprocals + normalise into ctxb. Returns (ctxb, ...)"""
        rcp = norms.tile([P, 2, kpc, 1], F32, tag="rcp", name="rcp")
        nc.vector.reciprocal(rcp[:, 0, :, :], pv[0][:, :, 64:65])
        nc.vector.reciprocal(rcp[:, 1, :, :], pv[1][:, :, 0:1])
        ctxb = norms.tile([P, kpc, P], F16, tag="ctxb", name="ctxb")
        for qs in range(kpc):
            nc.vector.tensor_scalar_mul(
                ctxb[:, qs, 0:64], pv[0][:, qs, 0:64], rcp[:, 0, qs, :])
            nc.vector.tensor_scalar_mul(
                ctxb[:, qs, 64:128], pv[1][:, qs, 1:65], rcp[:, 1, qs, :])
        if DBG and j == 0:
            nc.sync.dma_start(io["dbg_ctxb"][:, :, :], ctxb[:])
        return ctxb

    def postproc_steps(j, ctxb, borrow):
        """PE closures: 4 transposes (+evict), 8 Wo matmuls (+evict, dma).

        When `borrow` is set, every other Wo output goes to the pv0 bank,
        which sits idle from phase start until the current chunk's first PV
        group -- a 2-wide rotation that halves the matmul->evict bank chain.
        The po tiles must be created HERE (before the caller allocates the
        current chunk's pv tiles) so the pv0 tag generations stay ordered."""
        bb, jj = divmod(j, nch)
        steps = []
        ctx_t = norms.tile([P, QC], F16, tag="ctxt", name="ctxt")
        trb = pspool.tile([P, kpc, P], F16, tag="wob", name="trb")
        pos = [pspool.tile([P, QC], F32,
                           tag=("pv0" if (borrow and m % 2 == 1) else "wob"),
                           name="po")
               for m in range(nf)]
        for qs in range(kpc):
            def tr(qs=qs, ctxb=ctxb, trb=trb, ctx_t=ctx_t):
                nc.tensor.transpose(trb[:, qs, :], ctxb[:, qs, :], idn_sb[:])
                nc.vector.tensor_copy(ctx_t[:, qs * P:(qs + 1) * P],
                                      trb[:, qs, :])
            steps.append(tr)
        if DBG and j == 0:
            def dmp(ctx_t=ctx_t):
                nc.sync.dma_start(io["dbg_ctxt"][:, :], ctx_t[:])
            steps.append(dmp)
        wout = wouts.tile([P, nf, QC], F16, tag="wout", name="wout")
        for m in range(nf):
            def womm(m=m, ctx_t=ctx_t, wout=wout, bb=bb, jj=jj):
                po = pos[m]
                nc.tensor.matmul(po[:], wo_sb[:, m * P:(m + 1) * P], ctx_t[:],
                                 start=True, stop=True)
                nc.vector.tensor_copy(wout[:, m, :], po[:])
                if m == nf - 1:
                    nc.gpsimd.dma_start(
                        out_t.rearrange("(o p) t -> p o t", p=P)
                             [:, :, bb * s + jj * QC:bb * s + (jj + 1) * QC],
                        wout[:, :, :])
            steps.append(womm)
        return steps

    # ---- attention -------------------------------------------------------
    sw_parity = [0]

    def emit_qk_exp(bb, jj, t, qtile, pt_list):
        """QK matmuls + exp for k-tile t of chunk (bb, jj); stores the P^T
        tile (kept alive for the whole chunk) in pt_list."""
        jk = t // kpc
        ko = (t % kpc) * P
        tdiag = t - kpc * jj
        ktile = kt_tiles[(bb, jk)]
        tag = "swA" if sw_parity[0] == 0 else "swB"
        sw_parity[0] ^= 1
        sw = pspool.tile([P, 2 * QC], F32, tag=tag, name=tag)
        c0 = max(0, P * tdiag)
        if tdiag < 0:
            nc.tensor.matmul(sw[:, 0:QC], ktile[0:64, ko:ko + P],
                             qtile[0:64, :], start=True, stop=True)
            nc.tensor.matmul(sw[:, QC:2 * QC], ktile[64:128, ko:ko + P],
                             qtile[64:128, :], start=True, stop=True)
        else:
            # diagonal tile: add -60000 above the diagonal via a matmul
            # (stationary -60000*I, moving strict-lower-tri) so exp yields
            # exact zeros -- no separate mask multiply on a vector engine.
            # PSUM accumulation groups in a bank must be sequential, so each
            # 128-wide masked region closes before the next region opens.
            for h in range(HPC):
                lo = c0 if h == 0 else QC
                kk = ktile[h * 64:h * 64 + 64, ko:ko + P]
                qq = qtile[h * 64:h * 64 + 64, c0:QC]
                nc.tensor.matmul(sw[:, lo:lo + P], negi_sb[:],
                                 slt_sb[:], start=True, stop=False)
                nc.tensor.matmul(sw[:, lo:lo + P], kk, qq[:, 0:P],
                                 start=False, stop=True)
                if QC - c0 > P:
                    nc.tensor.matmul(sw[:, lo + P:lo + QC - c0], kk,
                                     qq[:, P:], start=True, stop=True)
        pt = pts.tile([P, 2 * QC], F16, tag="ptw", name="ptw")
        nc.scalar.activation(pt[:, c0:2 * QC - c0], sw[:, c0:2 * QC - c0],
                             Exp, scale=0.125)
        if DBG and bb == 0 and jj == 0 and t == 0:
            nc.sync.dma_start(io["dbg_pt"][:, :], pt[:])
        pt_list.append(pt)

    def emit_pv_group(bb, jj, qs, pt_list, pv):
        """All PV matmuls for q-subtile qs of both heads. A PSUM bank allows
        only one open accumulation group, so each (head, qs) group runs
        start..stop consecutively; tiles 0..kpc*jj+qs contribute."""
        tstop = kpc * jj + qs
        for h in range(HPC):
            for t in range(tstop + 1):
                tdiag = t - kpc * jj
                c0 = max(0, P * tdiag)
                vt = v_tiles[(bb, t)]
                pt = pt_list[t]
                mov = vt[:, 0:65] if h == 0 else vt[:, 64:129]
                if h == 0:
                    st = pt[:, qs * P:(qs + 1) * P]
                else:
                    st = pt[:, QC + qs * P - c0:QC + (qs + 1) * P - c0]
                nc.tensor.matmul(pv[h][:, qs, :], st, mov,
                                 start=(t == 0), stop=(t == tstop))

    # ---- main schedule ---------------------------------------------------
    prev = None  # (j, pv) of the chunk awaiting postprocessing
    pt_store = {}
    PULL = 2

    # chunk 0 Q/K projections: interleaved term-by-term with K in the (idle
    # at startup) wob bank, so both heads' main fp8 terms run as soon as the
    # hi-half DMAs land and neither waits on the other's eviction.  V weaves
    # into phase 0 as fillers.
    ppq0 = pspool.tile([P, QC], F32, tag="proj", name="ppq0")
    ppk0 = pspool.tile([P, QC], F32, tag="wob", name="ppk0")
    for ti, (ww, xw) in enumerate([(0, 0), (1, 0), (0, 1)]):
        for nm, pp in (("q", ppq0), ("k", ppk0)):
            for fp in range(nfp):
                nc.tensor.matmul(pp[:], w_sb[(nm, ww)][:, fp, :, :],
                                 xsl(0, nm, xw, fp, 0, QC),
                                 start=(ti == 0 and fp == 0),
                                 stop=(ti == 2 and fp == nfp - 1),
                                 perf_mode=DR)
    qt0 = qkpool.tile([P, QC], F16, tag="qt", name="qt")
    nc.scalar.activation(qt0[:], ppq0[:], Identity, bias=bq_sb[:, 0:1],
                         scale=1.0 / 32.0)
    qt_tiles[0] = qt0
    kt0 = persist.tile([P, QC], F16, tag="kt0", name="kt0")
    nc.scalar.activation(kt0[:], ppk0[:], Identity, bias=bk_sb[:, 0:1],
                         scale=1.0 / 32.0)
    kt_tiles[(0, 0)] = kt0
    carry0 = proj_steps(0, v_only=True)

    for j in range(nj):
        bb, jj = divmod(j, nch)
        if bb == 0 and jj >= 1:
            # prefetch batch-1 inputs in waves: early enough to clear
            # DMA_ENGINES before use, late enough that output DMAs are not
            # FIFO-ordered behind the whole input stream
            if jj == 1:
                issue_c0(1)
                issue_xt(1, [1])
            elif jj == 2:
                issue_xt(1, [2])
            else:
                issue_xt(1, [3])

        # fillers: postprocess chunk j-1, then project chunk j+1 (round-robin)
        head = carry0 if j == 0 else []
        fillers = []
        if prev is not None:
            pj, ppv = prev
            ctxb = postproc_prologue(pj, ppv)
            fillers.extend(postproc_steps(pj, ctxb, borrow=False))
            prev = None
        pf = proj_steps(j + 1) if j + 1 < nj else []
        # weighted merge (1 postproc : 2 proj): keeps >=2 independent proj
        # matmuls between Wo steps so the single-bank Wo matmul->evict chain
        # never stalls the in-order PE
        merged = []
        a, bidx = 0, 0
        while a < len(fillers) or bidx < len(pf):
            if a < len(fillers):
                merged.append(fillers[a]); a += 1
            for _ in range(3):
                if bidx < len(pf):
                    merged.append(pf[bidx]); bidx += 1
        fillers = head + merged

        ktiles = kpc * (jj + 1)
        pv = {0: pspool.tile([P, kpc, 65], F32, tag="pv0", name="pv0"),
              1: pspool.tile([P, kpc, 65], F32, tag="pv1", name="pv1")}
        qtile = qt_tiles[j]

        fi = [0]
        def weave(slot, ktiles=ktiles, fillers=fillers, fi=fi):
            want = ((slot + 1) * len(fillers) + ktiles - 1) // ktiles
            while fi[0] < min(want, len(fillers)):
                fillers[fi[0]]()
                fi[0] += 1

        pt_list = pt_store.pop(j, [])
        skip = len(pt_list)
        last = (j == nj - 1)
        if last:
            # tail pipeline: as each q-subtile's PV group completes, run its
            # norm/transpose/evict immediately; Wo afterwards cycles through
            # the now-free PSUM banks so its evictions never gate the PE.
            rcp_t = norms.tile([P, 2, kpc, 1], F32, tag="rcp", name="rcp")
            ctxb_t = norms.tile([P, kpc, P], F16, tag="ctxb", name="ctxb")
            ctxt_t = norms.tile([P, QC], F16, tag="ctxt", name="ctxt")
            trb_t = pspool.tile([P, kpc, P], F16, tag="proj", name="trb_t")
        def tail_chain(qs):
            nc.vector.reciprocal(rcp_t[:, 0, qs, :], pv[0][:, qs, 64:65])
            nc.vector.reciprocal(rcp_t[:, 1, qs, :], pv[1][:, qs, 0:1])
            nc.vector.tensor_scalar_mul(
                ctxb_t[:, qs, 0:64], pv[0][:, qs, 0:64], rcp_t[:, 0, qs, :])
            nc.vector.tensor_scalar_mul(
                ctxb_t[:, qs, 64:128], pv[1][:, qs, 1:65], rcp_t[:, 1, qs, :])
            nc.tensor.transpose(trb_t[:, qs, :], ctxb_t[:, qs, :], idn_sb[:])
            nc.vector.tensor_copy(ctxt_t[:, qs * P:(qs + 1) * P],
                                  trb_t[:, qs, :])

        # PV groups are emitted one k-tile later than their stop tile so the
        # exp they depend on has drained before the PE reaches them
        for t in range(skip, ktiles):
            emit_qk_exp(bb, jj, t, qtile, pt_list)
            weave(t)
            tdiag = t - kpc * jj
            if tdiag >= 1:
                emit_pv_group(bb, jj, tdiag - 1, pt_list, pv)
                if last:
                    tail_chain(tdiag - 1)
        emit_pv_group(bb, jj, kpc - 1, pt_list, pv)
        if last:
            tail_chain(kpc - 1)
        while fi[0] < len(fillers):
            fillers[fi[0]]()
            fi[0] += 1
        # pull forward the next chunk's full k-tiles (QK+exp only): spreads
        # the exp load of late, attention-heavy chunks into earlier phases
        if j + 1 < nj:
            bb2, jj2 = divmod(j + 1, nch)
            npull = min(PULL, kpc * jj2)
            pt_next = []
            for tt in range(npull):
                emit_qk_exp(bb2, jj2, tt, qt_tiles[j + 1], pt_next)
            pt_store[j + 1] = pt_next
        if last:
            # tail Wo: cycle now-free PSUM banks, alternate DVE/ACT evicts,
            # and ship the output in two halves so the DMA starts early
            wout = wouts.tile([P, nf, QC], F16, tag="wout", name="wout")
            out_r = out_t.rearrange("(o p) t -> p o t", p=P)
            cl, ch = bb * s + jj * QC, bb * s + (jj + 1) * QC
            for m in range(nf):
                tag = ("wob", "proj", "swA", "swB")[m % 4]
                po = pspool.tile([P, QC], F32, tag=tag, name="po")
                nc.tensor.matmul(po[:], wo_sb[:, m * P:(m + 1) * P],
                                 ctxt_t[:], start=True, stop=True)
                if m % 2 == 0:
                    nc.vector.tensor_copy(wout[:, m, :], po[:])
                else:
                    nc.scalar.copy(wout[:, m, :], po[:])
                if m % 2 == 1:
                    # quarter DMAs on the SP/HWDGE path: transfers pipeline
                    # with the remaining evicts and the last one is short
                    nc.sync.dma_start(out_r[:, m - 1:m + 1, cl:ch],
                                      wout[:, m - 1:m + 1, :])
            prev = None
        else:
            prev = (j, pv)


def build_program(s=S, d=D, b=B):
    import concourse.tile as tile
    from concourse import bacc, mybir
    from contextlib import ExitStack

    F16 = mybir.dt.float16
    F32 = mybir.dt.float32
    bs = b * s

    F8 = mybir.dt.float8e4
    nc = bacc.Bacc("TRN2", target_bir_lowering=False, debug=False)
    io = {
        "xq2": nc.dram_tensor("xq2", [2 * d, bs], F8, kind="ExternalInput").ap(),
        "xk2": nc.dram_tensor("xk2", [2 * d, bs], F8, kind="ExternalInput").ap(),
        "xv2": nc.dram_tensor("xv2", [2 * d, bs], F8, kind="ExternalInput").ap(),
        "wq2": nc.dram_tensor("wq2", [P, 2 * d], F8, kind="ExternalInput").ap(),
        "wk2": nc.dram_tensor("wk2", [P, 2 * d], F8, kind="ExternalInput").ap(),
        "wv2": nc.dram_tensor("wv2", [P, 2 * d], F8, kind="ExternalInput").ap(),
        "wo": nc.dram_tensor("wo", [P, d], F16, kind="ExternalInput").ap(),
        "bq": nc.dram_tensor("bq", [P, 1], F32, kind="ExternalInput").ap(),
        "bk": nc.dram_tensor("bk", [P, 1], F32, kind="ExternalInput").ap(),
        "tri": nc.dram_tensor("tri", [P, P], F16, kind="ExternalInput").ap(),
        "slt": nc.dram_tensor("slt", [P, P], F16, kind="ExternalInput").ap(),
        "idn": nc.dram_tensor("idn", [P, P], F16, kind="ExternalInput").ap(),
        "out_t": nc.dram_tensor("out_t", [d, bs], F16, kind="ExternalOutput").ap(),
    }
    if DBG:
        for nm, shp, dt in [("dbg_qt", [P, QC], F16), ("dbg_kt", [P, QC], F16),
                            ("dbg_vt", [P, 130], F16), ("dbg_pt", [P, 2 * QC], F16),
                            ("dbg_ctxb", [P, QC // P, P], F16),
                            ("dbg_ctxt", [P, QC], F16)]:
            io[nm] = nc.dram_tensor(nm, shp, dt, kind="ExternalOutput").ap()
    with tile.TileContext(nc) as tc, ExitStack() as ctx:
        _mha_body(ctx, tc, io, s, d, b)
    nc.compile()
    return nc


# ---------------------------------------------------------------------------
# Host side
# ---------------------------------------------------------------------------

def _np_reference(query, key, value, mask, Wq, bq, Wk, bk, Wv, bv, Wo, bo):
    """Pure-numpy fallback, exact reference math (used only if the mask is
    not the expected causal mask)."""
    q = (query.reshape(-1, D) @ Wq + bq).reshape(B, S, H, DK).transpose(0, 2, 1, 3)
    k = (key.reshape(-1, D) @ Wk + bk).reshape(B, S, H, DK).transpose(0, 2, 1, 3)
    v = (value.reshape(-1, D) @ Wv + bv).reshape(B, S, H, DK).transpose(0, 2, 1, 3)
    scores = np.einsum("bhqd,bhkd->bhqk", q, k) / math.sqrt(DK)
    scores = np.where(mask[:, None, :, :] == 0, np.float32(-1e9), scores)
    scores -= scores.max(axis=-1, keepdims=True)
    p = np.exp(scores)
    p /= p.sum(axis=-1, keepdims=True)
    x = np.einsum("bhqk,bhkd->bhqd", p, v)
    x = x.transpose(0, 2, 1, 3).reshape(B, -1, D)
    return (x @ Wo + bo).astype(np.float32)


def _f8():
    import ml_dtypes
    return ml_dtypes.float8_e4m3


def _q8(a):
    """Quantise to fp8 e4m3, returning (fp8 array, fp32 residual)."""
    f8 = _f8()
    q = np.asarray(a, np.float32).astype(f8)
    return q, np.asarray(a, np.float32) - q.astype(np.float32)


def _w8layout(w):
    """[D, 128] fp8 slice -> [128, D] with col = fp*256 + g*128 + m matching
    the device [P, nfp, 2, P] tile (row (2fp+g)*128+p)."""
    d = w.shape[0]
    nfp = d // P // 2
    return np.ascontiguousarray(
        w.reshape(nfp, 2, P, P).transpose(2, 0, 1, 3).reshape(P, d))


def _shard_inputs(query, key, value, Wq, bq, Wk, bk, Wv, Wo):
    f16 = np.float16
    f8 = _f8()
    xs8 = {}
    for nm, arr in (("q", query), ("k", key), ("v", value)):
        xt = np.ascontiguousarray(arr.reshape(B * S, D).T)
        hi, res = _q8(xt)
        xs8[nm] = np.concatenate([hi, res.astype(f8)], axis=0)
    idx = np.arange(P)
    tri = (-60000.0 * np.eye(P)).astype(f16)  # mask-add stationary
    slt = (idx[:, None] > idx[None, :]).astype(f16)  # strict lower tri
    idn = np.eye(P, dtype=f16)
    in_maps = []
    for c in range(NCORES):
        sl = slice(c * HPC * DK, (c + 1) * HPC * DK)
        m = {
            "xq2": xs8["q"], "xk2": xs8["k"], "xv2": xs8["v"],
            "wo": np.ascontiguousarray(Wo[sl, :]).astype(f16),
            "bq": np.ascontiguousarray(bq[sl]).reshape(P, 1).astype(np.float32),
            "bk": np.ascontiguousarray(bk[sl]).reshape(P, 1).astype(np.float32),
            "tri": tri,
            "slt": slt,
            "idn": idn,
        }
        for nm, W in (("q", Wq), ("k", Wk), ("v", Wv)):
            hi, res = _q8(32.0 * W[:, sl])
            m[f"w{nm}2"] = np.concatenate(
                [_w8layout(hi), _w8layout(res.astype(f8))], axis=1)
        in_maps.append(m)
    return in_maps


def kernel(**inputs):
    query = np.asarray(inputs["query"], np.float32)
    key = np.asarray(inputs["key"], np.float32)
    value = np.asarray(inputs["value"], np.float32)
    mask = np.asarray(inputs["mask"])
    Wq = np.asarray(inputs["Wq"], np.float32)
    bq = np.asarray(inputs["bq"], np.float32)
    Wk = np.asarray(inputs["Wk"], np.float32)
    bk = np.asarray(inputs["bk"], np.float32)
    Wv = np.asarray(inputs["Wv"], np.float32)
    bv = np.asarray(inputs["bv"], np.float32)
    Wo = np.asarray(inputs["Wo"], np.float32)
    bo = np.asarray(inputs["bo"], np.float32)

    # The device program hardcodes causal structure; fall back to exact host
    # math for any other mask.
    tril = np.tril(np.ones((S, S), np.int8))
    if mask.shape != (B, S, S) or not np.array_equal(
            (mask != 0).astype(np.int8), np.broadcast_to(tril, (B, S, S))):
        return _np_reference(query, key, value, mask,
                             Wq, bq, Wk, bk, Wv, bv, Wo, bo)

    in_maps = _shard_inputs(query, key, value, Wq, bq, Wk, bk, Wv, Wo)
    outs = _run_spmd(in_maps)

    acc = outs.astype(np.float32).sum(axis=0)  # [D, B*S]
    out = acc.T + (bv @ Wo + bo)[None, :]
    return out.reshape(B, S, D).astype(np.float32)


def _get_exec():
    """Build (once) the program + jitted SPMD executable."""
    if "exec" in _PROGRAM_CACHE:
        return _PROGRAM_CACHE["exec"]
    import jax
    from jax.sharding import Mesh, PartitionSpec
    from jax.experimental.shard_map import shard_map
    import concourse.mybir as mybir
    from concourse import bass2jax

    nc = build_program()
    _PROGRAM_CACHE["nc"] = nc
    bass2jax.install_neuronx_cc_hook()
    partition_name = nc.partition_id_tensor.name if nc.partition_id_tensor else None
    in_names, out_names, out_avals, zero_outs = [], [], [], []
    for alloc in nc.m.functions[0].allocations:
        if not isinstance(alloc, mybir.MemoryLocationSet):
            continue
        name = alloc.memorylocations[0].name
        if alloc.kind == "ExternalInput":
            if name != partition_name:
                in_names.append(name)
        elif alloc.kind == "ExternalOutput":
            out_names.append(name)
            shape = tuple(alloc.tensor_shape)
            dtype = mybir.dt.np(alloc.dtype)
            out_avals.append(jax.core.ShapedArray(shape, dtype))
            zero_outs.append(np.zeros(shape, dtype))
    n_params = len(in_names)
    all_in_names = list(in_names) + list(out_names)
    if partition_name is not None:
        all_in_names.append(partition_name)

    def _body(*args):
        operands = list(args)
        if partition_name is not None:
            operands.append(bass2jax.partition_id_tensor())
        return tuple(bass2jax._bass_exec_p.bind(
            *operands,
            out_avals=tuple(out_avals),
            in_names=tuple(all_in_names),
            out_names=tuple(out_names),
            lowering_input_output_aliases=(),
            sim_require_finite=True,
            sim_require_nnan=True,
            nc=nc,
        ))

    devices = jax.devices()[:NCORES]
    assert len(devices) >= NCORES, f"need {NCORES} neuron cores, have {len(devices)}"
    mesh = Mesh(np.asarray(devices[:NCORES]), ("core",))
    fn = jax.jit(
        shard_map(_body, mesh=mesh,
                  in_specs=(PartitionSpec("core"),) * (n_params + len(zero_outs)),
                  out_specs=(PartitionSpec("core"),) * len(out_names),
                  check_rep=False),
        donate_argnums=tuple(range(n_params, n_params + len(out_names))),
        keep_unused=True)
    _PROGRAM_CACHE["exec"] = (fn, in_names, zero_outs)
    return _PROGRAM_CACHE["exec"]


def _run_spmd(in_maps):
    """Run the SPMD program on 8 cores; returns per-core out_t [8, D, B*S]."""
    fn, in_names, zero_outs = _get_exec()
    concat_in = [np.concatenate([np.asarray(in_maps[c][nm])
                                 for c in range(NCORES)], axis=0)
                 for nm in in_names]
    concat_zero = [np.zeros((NCORES * z.shape[0], *z.shape[1:]), z.dtype)
                   for z in zero_outs]
    out = fn(*concat_in, *concat_zero)
    LAST["out"] = out
    return np.asarray(out[0]).reshape(NCORES, D, B * S)
